# revision 12
# baseline (speedup 1.0000x reference)
"""NonLocal block kernel for 8 Trainium2 NeuronCores.

Algebraic restructuring: the softmax-free attention

    s = theta^T phi / N ;  y = s . g^T   (per batch)

is reassociated as y = (G/N) @ theta with G[i,j] = sum_m g[i,m] phi[j,m]
(a [32,32] matrix per batch).  Folding the surrounding 1x1 convs:

    out = (I + W_w (G/N) theta_w) @ target + (W_w (G/N) theta_b + W_b)

so after G is known the whole module is one 64x64 1x1-conv over target.

Sharding: batch b -> core pair (2b, 2b+1); each core of the pair computes
G for its batch redundantly (reads full ref/ref_align for the batch) and
produces half of the spatial output (no cross-core communication).

Precision: tolerance is 2e-2 rel; the G path only perturbs the output at
the ~1e-3 level, so refs and the phi/g conv run in fp8e4 (weights scaled
x16 to dodge denormals; the x256 on G is divided out once).  target, the
final conv, and the output are bf16 (~0.4% rel), PSUM accumulation f32.
W4 is accumulated as the pure correction (W4 - I)^T; the identity is
re-added during the bf16 block-diag build (diag correction ~1e-4 is below
bf16 ulp either way).

Engine split: PE does convs/transposes/G (block-diagonal stationaries
fuse phi+g and both u-halves into single full-depth matmuls), DVE does
only the 2x2 maxpool reduces plus half the output bias-adds, ACT adds
the phi/g biases onto pooled (per-partition bias), evacuates transposed
groups from PSUM, and does the other half of output bias-adds.

All DRAM tensors are chunk-major (host repacked) so every DMA is one
fully contiguous block; all input DMAs are issued up-front to
persistent tiles across the SP/ACT/Pool queues.
"""

import sys

for _p in ("/opt/trn_rl_repo",):
    if _p not in sys.path:
        sys.path.insert(0, _p)

import ml_dtypes
import numpy as np

import concourse.bass as bass
import concourse.mybir as mybir
from concourse import bacc
import concourse.tile as tile
from concourse.masks import make_identity
from concourse.bass_utils import run_bass_kernel_spmd

B, C, IC, H, W = 4, 64, 32, 128, 128
N = H * W            # 16384
NH = N // 2          # spatial positions per core (half batch)
M = N // 4           # 4096 pooled positions per batch
FP32 = mybir.dt.float32
BF16 = mybir.dt.bfloat16
FP8 = mybir.dt.float8e4
SCALE = 16.0         # host scales pgw and pgb by this; /SCALE^2 at gt copy

# refs chunk column sizes (chunk-major in DRAM, one contiguous DMA each);
# front chunks are small so the first conv can start early
RSPLIT = (1024, 1024, 2048, 2048, 2048, 2048, 2048, 2048, 2048)
assert sum(RSPLIT) == N
TCHUNK = 2048        # tgt cols per chunk (bf16 -> 512 KiB contiguous)
OCHUNK = 1024        # out cols per write (bf16 -> 256 KiB contiguous)
# pooled-block grouping for transpose/G/W4 streaming: small tail groups
# shorten the post-stream drain
GROUPS = ((0, 4), (4, 4), (8, 4), (12, 2), (14, 2))

_CACHED = {}


def _build_program() -> bass.Bass:
    nc = bacc.Bacc("TRN2", target_bir_lowering=False, debug=False)

    refs = nc.dram_tensor("refs", [len(RSPLIT) * 128, 2048], FP8,
                          kind="ExternalInput")
    tgt = nc.dram_tensor("tgt", [2 * 128, TCHUNK], BF16, kind="ExternalInput")
    pgw = nc.dram_tensor("pgw", [128, C], FP8, kind="ExternalInput")
    wB = nc.dram_tensor("wB", [IC, 2 * C + 1], FP32, kind="ExternalInput")
    wbr = nc.dram_tensor("wbr", [1, C], FP32, kind="ExternalInput")
    pgb = nc.dram_tensor("pgb", [128, 1], FP32, kind="ExternalInput")
    out = nc.dram_tensor("o", [4 * 128, OCHUNK], BF16, kind="ExternalOutput")

    with tile.TileContext(nc) as tc, \
         nc.allow_low_precision("bf16/fp8 path well within 2e-2 tolerance"):
        with (
            tc.tile_pool(name="const", bufs=1) as cpool,
            tc.tile_pool(name="small", bufs=2) as sbS,
            tc.tile_pool(name="persist", bufs=1) as pers,
        ):
            # ---- all input DMAs issued up-front; every destination is
            # persistent so no trigger ever waits on a buffer recycle ----
            pgw_sb = cpool.tile([128, C], FP8, tag="pgw")
            refs_t = []
            for k, w in enumerate(RSPLIT):
                t = pers.tile([128, w], FP8, tag=f"refs{k}")
                refs_t.append(t)
            nc.sync.dma_start(out=pgw_sb[:], in_=pgw[:])
            for k in range(len(RSPLIT)):
                eng = nc.sync if k < 5 else nc.scalar
                eng.dma_start(
                    out=refs_t[k][:],
                    in_=refs[k * 128:(k + 1) * 128, 0:RSPLIT[k]],
                )
            wB_sb = cpool.tile([IC, 2 * C + 1], FP32, tag="wB")
            nc.scalar.dma_start(out=wB_sb[:], in_=wB[:])
            wbr_sb = cpool.tile([1, C], FP32, tag="wbr")
            nc.scalar.dma_start(out=wbr_sb[:], in_=wbr[:])
            pgb_sb = cpool.tile([128, 1], FP32, tag="pgb")
            nc.scalar.dma_start(out=pgb_sb[:], in_=pgb[:])
            # gpsimd: identity build first (transposes need it early),
            # then the target prefetch
            idbf_sb = cpool.tile([128, 128], BF16, tag="identb")
            make_identity(nc, idbf_sb[:])
            one_sb = cpool.tile([1, 1], FP32, tag="one")
            nc.gpsimd.memset(one_sb[:], 1.0)
            w4bd = pers.tile([128, 128], BF16, tag="w4bd")
            nc.gpsimd.memset(w4bd[:], 0.0)
            tgt_tiles = []
            for tq in range(2):
                tt = pers.tile([128, TCHUNK], BF16, tag=f"tgt{tq}")
                tgt_tiles.append(tt)
                nc.gpsimd.dma_start(
                    out=tt[:], in_=tgt[tq * 128:(tq + 1) * 128, :]
                )

            thw_sb = wB_sb[:, 0:C]
            wwT_sb = wB_sb[:, C:2 * C]
            thb_sb = wB_sb[:, 2 * C:2 * C + 1]

            # pooled conv outputs (bf16, x16 scaled, +bias), 4-way stacked
            pooled = pers.tile([128, 16 * 128], BF16, tag="pooled")
            # transposed pooled, bf16
            phigT = pers.tile([128, 16 * 128], BF16, tag="phigT")

            # map block index -> group index
            grp_of = {}
            for gi, (g0, gn) in enumerate(GROUPS):
                for b_ in range(g0, g0 + gn):
                    grp_of[b_] = gi
            NG = len(GROUPS)

            # ---- Phase A: fused fp8 convs + DVE 2x2 maxpool + ACT bias,
            # with transpose/G/W4 accumulation streamed per group ----
            with tc.tile_pool(name="psA", bufs=4, space="PSUM") as psA, \
                 tc.tile_pool(name="psB", bufs=2, space="PSUM") as psB, \
                 tc.tile_pool(name="psG", bufs=1, space="PSUM") as psG, \
                 tc.tile_pool(name="psW", bufs=1, space="PSUM") as psW:
                # W4 correction [cols 0:64] and v=G.theta_b [col 64] share
                # one PSUM bank (disjoint regions, independent accumulation)
                wv_ps = psW.tile([128, C + 1], FP32, tag="wv")
                w4_ps = wv_ps[:, 0:C]
                v_ps = wv_ps[0:IC, C:C + 1]
                tpp_box = [None]

                def emit_transpose(blk):
                    gi = grp_of[blk]
                    g0, gn = GROUPS[gi]
                    if blk == g0:
                        tpp_new = psB.tile([128, 128 * gn], BF16, tag="tp")
                        tpp_box[0] = tpp_new
                    bi = blk - g0
                    nc.tensor.matmul(
                        tpp_box[0][:, 128 * bi:128 * (bi + 1)],
                        pooled[:, 128 * blk:128 * (blk + 1)],
                        idbf_sb[:],
                        is_transpose=True, start=True, stop=True,
                        skip_group_check=True,
                    )

                def emit_group_tail(gi):
                    g0, gn = GROUPS[gi]
                    pT = phigT[:, 128 * g0:128 * (g0 + gn)]
                    # transposed group -> bf16 SBUF (plain ACT copy)
                    nc.scalar.activation(
                        pT, tpp_box[0][:], mybir.ActivationFunctionType.Copy
                    )
                    g_ps = psG.tile([IC, IC], FP32, tag="G")
                    for c in range(gn):
                        b0 = 128 * c
                        nc.tensor.matmul(
                            g_ps[:], pT[:, b0:b0 + IC],
                            pT[:, b0 + IC:b0 + 2 * IC],
                            start=(c == 0), stop=False,
                            skip_group_check=True,
                        )
                        nc.tensor.matmul(
                            g_ps[:], pT[:, b0 + 2 * IC:b0 + 3 * IC],
                            pT[:, b0 + 3 * IC:b0 + 4 * IC],
                            start=False, stop=(c == gn - 1),
                            skip_group_check=True,
                        )
                    gt_sb = sbS.tile([IC, IC], FP32, tag="Gt")
                    nc.scalar.activation(
                        gt_sb[:], g_ps[:],
                        mybir.ActivationFunctionType.Copy,
                        scale=1.0 / (N * SCALE * SCALE),
                    )
                    nc.tensor.matmul(v_ps, gt_sb[:], thb_sb,
                                     start=(gi == 0), stop=(gi == NG - 1),
                                     skip_group_check=True)
                    m2_ps = psG.tile([IC, C], FP32, tag="G")
                    nc.tensor.matmul(m2_ps[:], gt_sb[:], thw_sb,
                                     start=True, stop=True,
                                     skip_group_check=True)
                    m2_sb = sbS.tile([IC, C], FP32, tag="m2sb")
                    nc.scalar.activation(
                        m2_sb[:], m2_ps[:], mybir.ActivationFunctionType.Copy
                    )
                    for cpos in (0, 64):
                        nc.tensor.matmul(
                            w4_ps[cpos:cpos + C, :], m2_sb[:], wwT_sb,
                            start=(gi == 0), stop=(gi == NG - 1 and cpos == 64),
                            tile_position=(0, cpos),
                            skip_group_check=True,
                        )

                # cidx -> (refs tile, col offset) map
                cmap = []
                for k, w in enumerate(RSPLIT):
                    for off in range(0, w, 1024):
                        cmap.append((k, off))

                for cidx in range(16):
                    k, off = cmap[cidx]
                    rt = refs_t[k]
                    xs = slice(off, off + 512)
                    ys = slice(off + 512, off + 1024)
                    cp = psA.tile([128, 512], FP32, tag="conv")
                    # one full-depth matmul per 512-slice: block-diag pgw
                    # gives (phi; g) rows from (ref; ref_align) partitions
                    nc.tensor.matmul(cp[0:64, :], pgw_sb[:, 0:C], rt[:, xs],
                                     start=True, stop=True,
                                     tile_position=(0, 0))
                    nc.tensor.matmul(cp[64:128, :], pgw_sb[:, 0:C], rt[:, ys],
                                     start=True, stop=True,
                                     tile_position=(0, 64))
                    # fused 2x2 maxpool: windowed reduce, PSUM f32 -> bf16
                    po = pooled[:, cidx * 128:(cidx + 1) * 128]
                    nc.vector.reduce_max(
                        po.rearrange("p (hp w) -> p hp w", w=W // 2),
                        cp[:].rearrange("p (hp h0 w w0) -> p hp w h0 w0",
                                        hp=2, h0=2, w=W // 2, w0=2),
                        axis=mybir.AxisListType.XY,
                    )
                    # phi/g bias onto pooled: per-partition bias on ACT
                    # (bias commutes with maxpool)
                    nc.scalar.activation(
                        po, po, mybir.ActivationFunctionType.Identity,
                        bias=pgb_sb[:],
                    )
                    # transpose pipeline runs two chunks behind
                    prev = cidx - 2
                    if prev >= 0:
                        emit_transpose(prev)
                        gi = grp_of[prev]
                        if prev == GROUPS[gi][0] + GROUPS[gi][1] - 1:
                            emit_group_tail(gi)

                for prev in range(14, 16):
                    emit_transpose(prev)
                emit_group_tail(NG - 1)

                # block-diag (I + correction)^T on DVE (idle at drain)
                nc.vector.scalar_tensor_tensor(
                    out=w4bd[0:C, 0:C], in0=w4_ps[0:C, :], scalar=1.0,
                    in1=idbf_sb[0:C, 0:C],
                    op0=mybir.AluOpType.mult, op1=mybir.AluOpType.add,
                )
                nc.vector.scalar_tensor_tensor(
                    out=w4bd[C:128, C:128], in0=w4_ps[C:128, :], scalar=1.0,
                    in1=idbf_sb[0:C, 0:C],
                    op0=mybir.AluOpType.mult, op1=mybir.AluOpType.add,
                )
                v_sb = pers.tile([IC, 1], FP32, tag="vsb")
                nc.scalar.activation(
                    v_sb[:], v_ps, mybir.ActivationFunctionType.Copy
                )
                # b2 as a per-partition column, duplicated on partitions 64:128
                b2c_ps = psG.tile([128, 1], FP32, tag="G")
                for cpos in (0, 64):
                    nc.tensor.matmul(
                        b2c_ps[cpos:cpos + C, :], wwT_sb, v_sb[:],
                        start=True, stop=False, tile_position=(0, cpos),
                        skip_group_check=True,
                    )
                    nc.tensor.matmul(
                        b2c_ps[cpos:cpos + C, :], wbr_sb[:], one_sb[:, :],
                        start=False, stop=True, tile_position=(0, cpos),
                        skip_group_check=True,
                    )
                b2c_sb = pers.tile([128, 1], FP32, tag="b2csb")
                nc.scalar.activation(
                    b2c_sb[:], b2c_ps[:], mybir.ActivationFunctionType.Copy
                )

            # ---------- Phase D: fused final conv over target (bf16) ----------
            with tc.tile_pool(name="psD", bufs=4, space="PSUM") as psD, \
                 tc.tile_pool(name="outp", bufs=2) as sbO:
                for w in range(4):
                    tt = tgt_tiles[w // 2]
                    ot = sbO.tile([128, OCHUNK], BF16, tag="out")
                    for i in range(OCHUNK // 512):
                        op = psD.tile([128, 512], FP32, tag="od")
                        tsl = slice((w % 2) * OCHUNK + i * 512,
                                    (w % 2) * OCHUNK + (i + 1) * 512)
                        nc.tensor.matmul(
                            op[:], w4bd[:], tt[:, tsl],
                            start=True, stop=True,
                        )
                        # bias-add + bf16 cast, 256-col slices DVE/ACT
                        for q in range(2):
                            osl = slice(i * 512 + q * 256,
                                        i * 512 + (q + 1) * 256)
                            psl = slice(q * 256, (q + 1) * 256)
                            if q == 0:
                                nc.vector.tensor_scalar_add(
                                    ot[:, osl], op[:, psl], b2c_sb[:]
                                )
                            else:
                                nc.scalar.activation(
                                    ot[:, osl], op[:, psl],
                                    mybir.ActivationFunctionType.Identity,
                                    bias=b2c_sb[:],
                                )
                    eng = nc.sync if w % 2 == 0 else nc.scalar
                    eng.dma_start(
                        out=out[w * 128:(w + 1) * 128, :], in_=ot[:]
                    )

    nc.compile()
    return nc


def _in_maps(target, ref, ref_align, theta_w, theta_b, phi_w, phi_b,
             g_w, g_b, W_w, W_b):
    f32, bf16 = np.float32, ml_dtypes.bfloat16
    fp8 = ml_dtypes.float8_e4m3
    wBv = np.zeros((IC, 2 * C + 1), dtype=f32)
    wBv[:, 0:C] = theta_w
    wBv[:, C:2 * C] = W_w.T
    wBv[:, 2 * C] = theta_b
    # block-diag stationary: rows 0:64 -> phi_w.T in cols 0:32,
    # rows 64:128 -> g_w.T in cols 32:64
    pgwbd = np.zeros((128, C), dtype=f32)
    pgwbd[0:C, 0:IC] = phi_w.T * SCALE
    pgwbd[C:128, IC:C] = g_w.T * SCALE
    pgbc = np.concatenate([phi_b, g_b, phi_b, g_b]).reshape(128, 1) * SCALE
    common = {
        "pgw": pgwbd.astype(fp8),
        "wB": wBv,
        "wbr": W_b.reshape(1, C).astype(f32),
        "pgb": pgbc.astype(f32),
    }
    maps = []
    for core in range(8):
        b, u = core // 2, core % 2
        refs = np.concatenate(
            [ref[b].reshape(C, N), ref_align[b].reshape(C, N)], axis=0
        ).astype(fp8)
        # chunk-major repack, padded to 2048 cols per chunk row-block
        refsc = np.zeros((len(RSPLIT) * 128, 2048), dtype=fp8)
        col = 0
        for k, wd in enumerate(RSPLIT):
            refsc[k * 128:(k + 1) * 128, 0:wd] = refs[:, col:col + wd]
            col += wd
        th = target[b, :, u * (H // 2):(u + 1) * (H // 2), :].reshape(C, NH)
        tgtv = np.concatenate([th[:, :NH // 2], th[:, NH // 2:]], axis=0)
        tgtc = np.concatenate(
            [tgtv[:, q * TCHUNK:(q + 1) * TCHUNK] for q in range(2)], axis=0
        ).astype(bf16)
        maps.append({"refs": refsc,
                     "tgt": np.ascontiguousarray(tgtc), **common})
    return maps


def _gather(res) -> np.ndarray:
    out = np.empty((B, C, H, W), dtype=np.float32)
    for core in range(8):
        o = np.asarray(res.results[core]["o"]).astype(np.float32)
        # undo chunk-major: [4*128, 1024] -> [128, 4096] -> [64, 8192]
        o = np.concatenate([o[w * 128:(w + 1) * 128, :] for w in range(4)],
                           axis=1)
        half = np.concatenate([o[:C, :], o[C:, :]], axis=1)  # [64, 8192]
        b, u = core // 2, core % 2
        out[b, :, u * (H // 2):(u + 1) * (H // 2), :] = half.reshape(C, H // 2, W)
    return out


def kernel(**inputs) -> np.ndarray:
    if "nc" not in _CACHED:
        _CACHED["nc"] = _build_program()
    nc = _CACHED["nc"]
    maps = _in_maps(**inputs)
    res = run_bass_kernel_spmd(nc, maps, list(range(8)))
    return _gather(res)


# revision 19
# speedup vs baseline: 1.0428x; 1.0428x over previous
"""NonLocal block kernel for 8 Trainium2 NeuronCores.

Algebraic restructuring: the softmax-free attention

    s = theta^T phi / N ;  y = s . g^T   (per batch)

is reassociated as y = (G/N) @ theta with G[i,j] = sum_m g[i,m] phi[j,m]
(a [32,32] matrix per batch).  Folding the surrounding 1x1 convs:

    out = (I + W_w (G/N) theta_w) @ target + (W_w (G/N) theta_b + W_b)

so after G is known the whole module is one 64x64 1x1-conv over target.

Sharding: batch b -> core pair (2b, 2b+1); each core of the pair computes
G for its batch redundantly (reads full ref/ref_align for the batch) and
produces half of the spatial output (no cross-core communication).

Precision: tolerance is 2e-2 rel; the G path only perturbs the output at
the ~1e-3 level, so refs and the phi/g conv run in fp8e4 (weights scaled
x16 to dodge denormals; the x256 on G is divided out once).  target, the
final conv, and the output are bf16 (~0.4% rel), PSUM accumulation f32.
W4 is accumulated as the pure correction (W4 - I)^T; the identity is
re-added during the bf16 block-diag build (diag correction ~1e-4 is below
bf16 ulp either way).

Engine split: PE does convs/transposes/G (block-diagonal stationaries
fuse phi+g and both u-halves into single full-depth matmuls), DVE does
only the 2x2 maxpool reduces plus half the output bias-adds, ACT adds
the phi/g biases onto pooled (per-partition bias), evacuates transposed
groups from PSUM, and does the other half of output bias-adds.

All DRAM tensors are chunk-major (host repacked) so every DMA is one
fully contiguous block; all input DMAs are issued up-front to
persistent tiles across the SP/ACT/Pool queues.
"""

import sys

for _p in ("/opt/trn_rl_repo",):
    if _p not in sys.path:
        sys.path.insert(0, _p)

import ml_dtypes
import numpy as np

import concourse.bass as bass
import concourse.mybir as mybir
from concourse import bacc
import concourse.tile as tile
from concourse.masks import make_identity
from concourse.bass_utils import run_bass_kernel_spmd

B, C, IC, H, W = 4, 64, 32, 128, 128
N = H * W            # 16384
NH = N // 2          # spatial positions per core (half batch)
M = N // 4           # 4096 pooled positions per batch
FP32 = mybir.dt.float32
BF16 = mybir.dt.bfloat16
FP8 = mybir.dt.float8e4
SCALE = 16.0         # host scales pgw and pgb by this; /SCALE^2 at gt copy

# refs chunk column sizes (chunk-major in DRAM, one contiguous DMA each);
# front chunks are small so the first conv can start early
RSPLIT = (1024, 1024, 2048, 2048, 2048, 2048, 2048, 2048, 2048)
assert sum(RSPLIT) == N
TCHUNK = 2048        # tgt cols per chunk (bf16 -> 512 KiB contiguous)
OCHUNK = 1024        # out cols per write (bf16 -> 256 KiB contiguous)
# pooled-block grouping for transpose/G/W4 streaming: small tail groups
# shorten the post-stream drain
GROUPS = ((0, 4), (4, 4), (8, 4), (12, 2), (14, 2))

_CACHED = {}


def _build_program() -> bass.Bass:
    nc = bacc.Bacc("TRN2", target_bir_lowering=False, debug=False)

    refs = nc.dram_tensor("refs", [len(RSPLIT) * 128, 2048], FP8,
                          kind="ExternalInput")
    tgt = nc.dram_tensor("tgt", [2 * 128, TCHUNK], BF16, kind="ExternalInput")
    pgw = nc.dram_tensor("pgw", [128, C], FP8, kind="ExternalInput")
    wB = nc.dram_tensor("wB", [IC, 2 * C + 1], FP32, kind="ExternalInput")
    wbr = nc.dram_tensor("wbr", [1, C], FP32, kind="ExternalInput")
    biasT = nc.dram_tensor("biasT", [128, 512], BF16, kind="ExternalInput")
    out = nc.dram_tensor("o", [4 * 128, OCHUNK], BF16, kind="ExternalOutput")

    with tile.TileContext(nc) as tc, \
         nc.allow_low_precision("bf16/fp8 path well within 2e-2 tolerance"):
        with (
            tc.tile_pool(name="const", bufs=1) as cpool,
            tc.tile_pool(name="small", bufs=2) as sbS,
            tc.tile_pool(name="persist", bufs=1) as pers,
        ):
            # ---- all input DMAs issued up-front; every destination is
            # persistent so no trigger ever waits on a buffer recycle ----
            pgw_sb = cpool.tile([128, C], FP8, tag="pgw")
            refs_t = []
            for k, w in enumerate(RSPLIT):
                t = pers.tile([128, w], FP8, tag=f"refs{k}")
                refs_t.append(t)
            nc.sync.dma_start(out=pgw_sb[:], in_=pgw[:])
            for k in range(len(RSPLIT)):
                eng = nc.sync if k < 5 else nc.scalar
                eng.dma_start(
                    out=refs_t[k][:],
                    in_=refs[k * 128:(k + 1) * 128, 0:RSPLIT[k]],
                )
            wB_sb = cpool.tile([IC, 2 * C + 1], FP32, tag="wB")
            nc.scalar.dma_start(out=wB_sb[:], in_=wB[:])
            wbr_sb = cpool.tile([1, C], FP32, tag="wbr")
            nc.scalar.dma_start(out=wbr_sb[:], in_=wbr[:])
            biasT_sb = cpool.tile([128, 512], BF16, tag="biasT")
            nc.scalar.dma_start(out=biasT_sb[:], in_=biasT[:])
            # gpsimd: identity build first (transposes need it early),
            # then the target prefetch
            idbf_sb = cpool.tile([128, 128], BF16, tag="identb")
            make_identity(nc, idbf_sb[:])
            one_sb = cpool.tile([1, 1], FP32, tag="one")
            nc.gpsimd.memset(one_sb[:], 1.0)
            w4bd = pers.tile([128, 128], BF16, tag="w4bd")
            nc.gpsimd.memset(w4bd[:], 0.0)
            tgt_tiles = []
            for tq in range(2):
                tt = pers.tile([128, TCHUNK], BF16, tag=f"tgt{tq}")
                tgt_tiles.append(tt)
                nc.gpsimd.dma_start(
                    out=tt[:], in_=tgt[tq * 128:(tq + 1) * 128, :]
                )

            thw_sb = wB_sb[:, 0:C]
            wwT_sb = wB_sb[:, C:2 * C]
            thb_sb = wB_sb[:, 2 * C:2 * C + 1]

            # pooled conv outputs (bf16, x16 scaled, +bias), 4-way stacked
            pooled = pers.tile([128, 16 * 128], BF16, tag="pooled")
            # transposed pooled, bf16
            phigT = pers.tile([128, 16 * 128], BF16, tag="phigT")

            # map block index -> group index
            grp_of = {}
            for gi, (g0, gn) in enumerate(GROUPS):
                for b_ in range(g0, g0 + gn):
                    grp_of[b_] = gi
            NG = len(GROUPS)

            # ---- Phase A: fused fp8 convs + DVE 2x2 maxpool + ACT bias,
            # with transpose/G/W4 accumulation streamed per group ----
            with tc.tile_pool(name="psA", bufs=2, space="PSUM") as psA, \
                 tc.tile_pool(name="psB", bufs=2, space="PSUM") as psB, \
                 tc.tile_pool(name="psG", bufs=1, space="PSUM") as psG, \
                 tc.tile_pool(name="psW", bufs=1, space="PSUM") as psW:
                # W4 correction [cols 0:64] and v=G.theta_b [col 64] share
                # one PSUM bank (disjoint regions, independent accumulation)
                wv_ps = psW.tile([128, C + 1], FP32, tag="wv")
                w4_ps = wv_ps[:, 0:C]
                v_ps = wv_ps[0:IC, C:C + 1]
                tpp_box = [None]

                def emit_transpose(blk):
                    gi = grp_of[blk]
                    g0, gn = GROUPS[gi]
                    if blk == g0:
                        tpp_new = psB.tile([128, 128 * gn], BF16, tag="tp")
                        tpp_box[0] = tpp_new
                    bi = blk - g0
                    nc.tensor.matmul(
                        tpp_box[0][:, 128 * bi:128 * (bi + 1)],
                        pooled[:, 128 * blk:128 * (blk + 1)],
                        idbf_sb[:],
                        is_transpose=True, start=True, stop=True,
                        skip_group_check=True,
                    )

                def emit_group_tail(gi):
                    g0, gn = GROUPS[gi]
                    pT = phigT[:, 128 * g0:128 * (g0 + gn)]
                    # transposed group -> bf16 SBUF, phi/g bias fused into
                    # the copy (bias commutes with maxpool and transpose)
                    nc.vector.scalar_tensor_tensor(
                        out=pT, in0=tpp_box[0][:], scalar=1.0,
                        in1=biasT_sb[:, 0:128 * gn],
                        op0=mybir.AluOpType.mult, op1=mybir.AluOpType.add,
                    )
                    g_ps = psG.tile([IC, IC], FP32, tag="G")
                    for c in range(gn):
                        b0 = 128 * c
                        nc.tensor.matmul(
                            g_ps[:], pT[:, b0:b0 + IC],
                            pT[:, b0 + IC:b0 + 2 * IC],
                            start=(c == 0), stop=False,
                            skip_group_check=True,
                        )
                        nc.tensor.matmul(
                            g_ps[:], pT[:, b0 + 2 * IC:b0 + 3 * IC],
                            pT[:, b0 + 3 * IC:b0 + 4 * IC],
                            start=False, stop=(c == gn - 1),
                            skip_group_check=True,
                        )
                    gt_sb = sbS.tile([IC, IC], FP32, tag="Gt")
                    nc.scalar.activation(
                        gt_sb[:], g_ps[:],
                        mybir.ActivationFunctionType.Copy,
                        scale=1.0 / (N * SCALE * SCALE),
                    )
                    nc.tensor.matmul(v_ps, gt_sb[:], thb_sb,
                                     start=(gi == 0), stop=(gi == NG - 1),
                                     skip_group_check=True)
                    m2_ps = psG.tile([IC, C], FP32, tag="G")
                    nc.tensor.matmul(m2_ps[:], gt_sb[:], thw_sb,
                                     start=True, stop=True,
                                     skip_group_check=True)
                    m2_sb = sbS.tile([IC, C], FP32, tag="m2sb")
                    nc.scalar.activation(
                        m2_sb[:], m2_ps[:], mybir.ActivationFunctionType.Copy
                    )
                    for cpos in (0, 64):
                        nc.tensor.matmul(
                            w4_ps[cpos:cpos + C, :], m2_sb[:], wwT_sb,
                            start=(gi == 0), stop=(gi == NG - 1 and cpos == 64),
                            tile_position=(0, cpos),
                            skip_group_check=True,
                        )

                # 512-col slice -> (refs tile, col offset) map
                smap = []
                for k, w in enumerate(RSPLIT):
                    for off in range(0, w, 512):
                        smap.append((k, off))

                # process 2048 refs cols (2 pooled blocks) per PSUM tile:
                # one double-width DVE reduce per 4 conv matmuls
                for u in range(8):
                    cp = psA.tile([128, 1024], FP32, tag="conv")
                    for s in range(4):
                        k, off = smap[4 * u + s]
                        rt = refs_t[k]
                        # block-diag pgw gives (phi; g) rows from
                        # (ref; ref_align) partitions in one matmul
                        nc.tensor.matmul(
                            cp[64 * (s % 2):64 * (s % 2) + 64,
                               512 * (s // 2):512 * (s // 2) + 512],
                            pgw_sb[:, 0:C], rt[:, off:off + 512],
                            start=True, stop=True,
                            tile_position=(0, 64 * (s % 2)),
                        )
                    # fused 2x2 maxpool over both blocks: PSUM f32 -> bf16
                    po = pooled[:, u * 256:(u + 1) * 256]
                    nc.vector.reduce_max(
                        po.rearrange("p (q w) -> p q w", q=4),
                        cp[:].rearrange("p (q h0 w w0) -> p q w h0 w0",
                                        q=4, h0=2, w=W // 2, w0=2),
                        axis=mybir.AxisListType.XY,
                    )
                    # transpose pipeline runs one unit (2 blocks) behind
                    if u >= 1:
                        for prev in (2 * u - 2, 2 * u - 1):
                            emit_transpose(prev)
                            gi = grp_of[prev]
                            if prev == GROUPS[gi][0] + GROUPS[gi][1] - 1:
                                emit_group_tail(gi)

                for prev in (14, 15):
                    emit_transpose(prev)
                emit_group_tail(NG - 1)

                # block-diag (I + correction)^T on DVE (idle at drain)
                nc.vector.scalar_tensor_tensor(
                    out=w4bd[0:C, 0:C], in0=w4_ps[0:C, :], scalar=1.0,
                    in1=idbf_sb[0:C, 0:C],
                    op0=mybir.AluOpType.mult, op1=mybir.AluOpType.add,
                )
                nc.vector.scalar_tensor_tensor(
                    out=w4bd[C:128, C:128], in0=w4_ps[C:128, :], scalar=1.0,
                    in1=idbf_sb[0:C, 0:C],
                    op0=mybir.AluOpType.mult, op1=mybir.AluOpType.add,
                )
                v_sb = pers.tile([IC, 1], FP32, tag="vsb")
                nc.scalar.activation(
                    v_sb[:], v_ps, mybir.ActivationFunctionType.Copy
                )
                # b2 as a per-partition column, duplicated on partitions 64:128
                b2c_ps = psG.tile([128, 1], FP32, tag="G")
                for cpos in (0, 64):
                    nc.tensor.matmul(
                        b2c_ps[cpos:cpos + C, :], wwT_sb, v_sb[:],
                        start=True, stop=False, tile_position=(0, cpos),
                        skip_group_check=True,
                    )
                    nc.tensor.matmul(
                        b2c_ps[cpos:cpos + C, :], wbr_sb[:], one_sb[:, :],
                        start=False, stop=True, tile_position=(0, cpos),
                        skip_group_check=True,
                    )
                b2c_sb = pers.tile([128, 1], FP32, tag="b2csb")
                nc.scalar.activation(
                    b2c_sb[:], b2c_ps[:], mybir.ActivationFunctionType.Copy
                )

            # ---------- Phase D: fused final conv over target (bf16) ----------
            with tc.tile_pool(name="psD", bufs=4, space="PSUM") as psD, \
                 tc.tile_pool(name="outp", bufs=2) as sbO:
                for w in range(4):
                    tt = tgt_tiles[w // 2]
                    ot = sbO.tile([128, OCHUNK], BF16, tag="out")
                    for i in range(OCHUNK // 512):
                        op = psD.tile([128, 512], FP32, tag="od")
                        tsl = slice((w % 2) * OCHUNK + i * 512,
                                    (w % 2) * OCHUNK + (i + 1) * 512)
                        nc.tensor.matmul(
                            op[:], w4bd[:], tt[:, tsl],
                            start=True, stop=True,
                        )
                        # bias-add + bf16 cast, alternating DVE / ACT
                        isl = slice(i * 512, (i + 1) * 512)
                        if i % 2 == 0:
                            nc.vector.tensor_scalar_add(
                                ot[:, isl], op[:], b2c_sb[:]
                            )
                        else:
                            nc.scalar.activation(
                                ot[:, isl], op[:],
                                mybir.ActivationFunctionType.Identity,
                                bias=b2c_sb[:],
                            )
                    eng = nc.sync if w % 2 == 0 else nc.scalar
                    eng.dma_start(
                        out=out[w * 128:(w + 1) * 128, :], in_=ot[:]
                    )

    nc.compile()
    return nc


def _in_maps(target, ref, ref_align, theta_w, theta_b, phi_w, phi_b,
             g_w, g_b, W_w, W_b):
    f32, bf16 = np.float32, ml_dtypes.bfloat16
    fp8 = ml_dtypes.float8_e4m3
    wBv = np.zeros((IC, 2 * C + 1), dtype=f32)
    wBv[:, 0:C] = theta_w
    wBv[:, C:2 * C] = W_w.T
    wBv[:, 2 * C] = theta_b
    # block-diag stationary: rows 0:64 -> phi_w.T in cols 0:32,
    # rows 64:128 -> g_w.T in cols 32:64
    pgwbd = np.zeros((128, C), dtype=f32)
    pgwbd[0:C, 0:IC] = phi_w.T * SCALE
    pgwbd[C:128, IC:C] = g_w.T * SCALE
    bias512 = np.tile(np.concatenate([phi_b, g_b]) * SCALE, 8).reshape(1, 512)
    common = {
        "pgw": pgwbd.astype(fp8),
        "wB": wBv,
        "wbr": W_b.reshape(1, C).astype(f32),
        "biasT": np.broadcast_to(bias512, (128, 512)).astype(bf16),
    }
    maps = []
    for core in range(8):
        b, u = core // 2, core % 2
        refs = np.concatenate(
            [ref[b].reshape(C, N), ref_align[b].reshape(C, N)], axis=0
        ).astype(fp8)
        # chunk-major repack, padded to 2048 cols per chunk row-block
        refsc = np.zeros((len(RSPLIT) * 128, 2048), dtype=fp8)
        col = 0
        for k, wd in enumerate(RSPLIT):
            refsc[k * 128:(k + 1) * 128, 0:wd] = refs[:, col:col + wd]
            col += wd
        th = target[b, :, u * (H // 2):(u + 1) * (H // 2), :].reshape(C, NH)
        tgtv = np.concatenate([th[:, :NH // 2], th[:, NH // 2:]], axis=0)
        tgtc = np.concatenate(
            [tgtv[:, q * TCHUNK:(q + 1) * TCHUNK] for q in range(2)], axis=0
        ).astype(bf16)
        maps.append({"refs": refsc,
                     "tgt": np.ascontiguousarray(tgtc), **common})
    return maps


def _gather(res) -> np.ndarray:
    out = np.empty((B, C, H, W), dtype=np.float32)
    for core in range(8):
        o = np.asarray(res.results[core]["o"]).astype(np.float32)
        # undo chunk-major: [4*128, 1024] -> [128, 4096] -> [64, 8192]
        o = np.concatenate([o[w * 128:(w + 1) * 128, :] for w in range(4)],
                           axis=1)
        half = np.concatenate([o[:C, :], o[C:, :]], axis=1)  # [64, 8192]
        b, u = core // 2, core % 2
        out[b, :, u * (H // 2):(u + 1) * (H // 2), :] = half.reshape(C, H // 2, W)
    return out


def kernel(**inputs) -> np.ndarray:
    if "nc" not in _CACHED:
        _CACHED["nc"] = _build_program()
    nc = _CACHED["nc"]
    maps = _in_maps(**inputs)
    res = run_bass_kernel_spmd(nc, maps, list(range(8)))
    return _gather(res)


# revision 23
# speedup vs baseline: 1.0685x; 1.0247x over previous
"""NonLocal block kernel for 8 Trainium2 NeuronCores.

Algebraic restructuring: the softmax-free attention

    s = theta^T phi / N ;  y = s . g^T   (per batch)

is reassociated as y = (G/N) @ theta with G[i,j] = sum_m g[i,m] phi[j,m]
(a [32,32] matrix per batch).  Folding the surrounding 1x1 convs:

    out = (I + W_w (G/N) theta_w) @ target + (W_w (G/N) theta_b + W_b)

so after G is known the whole module is one 64x64 1x1-conv over target.

Sharding: batch b -> core pair (2b, 2b+1); each core of the pair computes
G for its batch redundantly (reads full ref/ref_align for the batch) and
produces half of the spatial output (no cross-core communication).

Precision: tolerance is 2e-2 rel; the G path only perturbs the output at
the ~1e-3 level, so refs and the phi/g conv run in fp8e4 (weights scaled
x16 to dodge denormals; the x256 on G is divided out once).  target, the
final conv, and the output are bf16 (~0.4% rel), PSUM accumulation f32.
W4 is accumulated as the pure correction (W4 - I)^T; the identity is
re-added during the bf16 block-diag build (diag correction ~1e-4 is below
bf16 ulp either way).

Engine split: PE does convs/transposes/G (block-diagonal stationaries
fuse phi+g and both u-halves into single full-depth matmuls), DVE does
only the 2x2 maxpool reduces plus half the output bias-adds, ACT adds
the phi/g biases onto pooled (per-partition bias), evacuates transposed
groups from PSUM, and does the other half of output bias-adds.

All DRAM tensors are chunk-major (host repacked) so every DMA is one
fully contiguous block; all input DMAs are issued up-front to
persistent tiles across the SP/ACT/Pool queues.
"""

import sys

for _p in ("/opt/trn_rl_repo",):
    if _p not in sys.path:
        sys.path.insert(0, _p)

import ml_dtypes
import numpy as np

import concourse.bass as bass
import concourse.mybir as mybir
from concourse import bacc
import concourse.tile as tile
from concourse.masks import make_identity
from concourse.bass_utils import run_bass_kernel_spmd

B, C, IC, H, W = 4, 64, 32, 128, 128
N = H * W            # 16384
NH = N // 2          # spatial positions per core (half batch)
M = N // 4           # 4096 pooled positions per batch
FP32 = mybir.dt.float32
BF16 = mybir.dt.bfloat16
FP8 = mybir.dt.float8e4
SCALE = 16.0         # host scales pgw and pgb by this; /SCALE^2 at gt copy

# refs chunk column sizes (chunk-major in DRAM, one contiguous DMA each);
# front chunks are small so the first conv can start early
RSPLIT = (1024, 1024, 2048, 2048, 2048, 2048, 2048, 2048, 2048)
assert sum(RSPLIT) == N
TCHUNK = 2048        # tgt cols per chunk (bf16 -> 512 KiB contiguous)
OCHUNK = 2048        # out cols per write (bf16 -> 512 KiB contiguous)
# pooled-block grouping for transpose/G/W4 streaming: small tail groups
# shorten the post-stream drain
GROUPS = ((0, 4), (4, 4), (8, 4), (12, 2), (14, 2))

_CACHED = {}


def _build_program() -> bass.Bass:
    nc = bacc.Bacc("TRN2", target_bir_lowering=False, debug=False)

    refs = nc.dram_tensor("refs", [len(RSPLIT) * 128, 2048], FP8,
                          kind="ExternalInput")
    tgt = nc.dram_tensor("tgt", [2 * 128, TCHUNK], BF16, kind="ExternalInput")
    pgw = nc.dram_tensor("pgw", [128, C], FP8, kind="ExternalInput")
    wB = nc.dram_tensor("wB", [IC, 2 * C + 1], FP32, kind="ExternalInput")
    wbr = nc.dram_tensor("wbr", [1, C], FP32, kind="ExternalInput")
    biasT = nc.dram_tensor("biasT", [128, 512], BF16, kind="ExternalInput")
    out = nc.dram_tensor("o", [2 * 128, OCHUNK], BF16, kind="ExternalOutput")

    with tile.TileContext(nc) as tc, \
         nc.allow_low_precision("bf16/fp8 path well within 2e-2 tolerance"):
        with (
            tc.tile_pool(name="const", bufs=1) as cpool,
            tc.tile_pool(name="small", bufs=2) as sbS,
            tc.tile_pool(name="persist", bufs=1) as pers,
        ):
            # ---- all input DMAs issued up-front; every destination is
            # persistent so no trigger ever waits on a buffer recycle ----
            pgw_sb = cpool.tile([128, C], FP8, tag="pgw")
            refs_t = []
            for k, w in enumerate(RSPLIT):
                t = pers.tile([128, w], FP8, tag=f"refs{k}")
                refs_t.append(t)
            # all refs strictly FIFO on the sync queue: a single queue still
            # saturates the fabric, and FIFO order means chunk k completes
            # before k+1 starts, so the conv is never starved by later bulk
            nc.sync.dma_start(out=pgw_sb[:], in_=pgw[:])
            for k in range(len(RSPLIT)):
                nc.sync.dma_start(
                    out=refs_t[k][:],
                    in_=refs[k * 128:(k + 1) * 128, 0:RSPLIT[k]],
                )
            wB_sb = cpool.tile([IC, 2 * C + 1], FP32, tag="wB")
            nc.scalar.dma_start(out=wB_sb[:], in_=wB[:])
            wbr_sb = cpool.tile([1, C], FP32, tag="wbr")
            nc.scalar.dma_start(out=wbr_sb[:], in_=wbr[:])
            biasT_sb = cpool.tile([128, 512], BF16, tag="biasT")
            nc.scalar.dma_start(out=biasT_sb[:], in_=biasT[:])
            # gpsimd: identity build first (transposes need it early),
            # then the target prefetch
            idbf_sb = cpool.tile([128, 128], BF16, tag="identb")
            make_identity(nc, idbf_sb[:])
            one_sb = cpool.tile([1, 1], FP32, tag="one")
            nc.gpsimd.memset(one_sb[:], 1.0)
            w4bd = pers.tile([128, 128], BF16, tag="w4bd")
            nc.gpsimd.memset(w4bd[:], 0.0)
            # tgt tiles allocated here; their DMAs are issued mid-stream
            # (on the scalar queue) so they don't steal refs bandwidth
            tgt_tiles = []
            for tq in range(2):
                tt = pers.tile([128, TCHUNK], BF16, tag=f"tgt{tq}")
                tgt_tiles.append(tt)

            thw_sb = wB_sb[:, 0:C]
            wwT_sb = wB_sb[:, C:2 * C]
            thb_sb = wB_sb[:, 2 * C:2 * C + 1]

            # pooled conv outputs (bf16, x16 scaled, +bias), 4-way stacked
            pooled = pers.tile([128, 16 * 128], BF16, tag="pooled")
            # transposed pooled, bf16
            phigT = pers.tile([128, 16 * 128], BF16, tag="phigT")

            # map block index -> group index
            grp_of = {}
            for gi, (g0, gn) in enumerate(GROUPS):
                for b_ in range(g0, g0 + gn):
                    grp_of[b_] = gi
            NG = len(GROUPS)

            # ---- Phase A: fused fp8 convs + DVE 2x2 maxpool + ACT bias,
            # with transpose/G/W4 accumulation streamed per group ----
            with tc.tile_pool(name="psA", bufs=2, space="PSUM") as psA, \
                 tc.tile_pool(name="psB", bufs=2, space="PSUM") as psB, \
                 tc.tile_pool(name="psG", bufs=1, space="PSUM") as psG, \
                 tc.tile_pool(name="psW", bufs=1, space="PSUM") as psW:
                # W4 correction [cols 0:64] and v=G.theta_b [col 64] share
                # one PSUM bank (disjoint regions, independent accumulation)
                wv_ps = psW.tile([128, C + 1], FP32, tag="wv")
                w4_ps = wv_ps[:, 0:C]
                v_ps = wv_ps[0:IC, C:C + 1]
                tpp_box = [None]

                def emit_transpose(blk):
                    gi = grp_of[blk]
                    g0, gn = GROUPS[gi]
                    if blk == g0:
                        tpp_new = psB.tile([128, 128 * gn], BF16, tag="tp")
                        tpp_box[0] = tpp_new
                    bi = blk - g0
                    nc.tensor.matmul(
                        tpp_box[0][:, 128 * bi:128 * (bi + 1)],
                        pooled[:, 128 * blk:128 * (blk + 1)],
                        idbf_sb[:],
                        is_transpose=True, start=True, stop=True,
                        skip_group_check=True,
                    )

                def emit_group_tail(gi):
                    g0, gn = GROUPS[gi]
                    pT = phigT[:, 128 * g0:128 * (g0 + gn)]
                    # transposed group -> bf16 SBUF, phi/g bias fused into
                    # the copy (bias commutes with maxpool and transpose)
                    nc.vector.scalar_tensor_tensor(
                        out=pT, in0=tpp_box[0][:], scalar=1.0,
                        in1=biasT_sb[:, 0:128 * gn],
                        op0=mybir.AluOpType.mult, op1=mybir.AluOpType.add,
                    )
                    g_ps = psG.tile([IC, IC], FP32, tag="G")
                    for c in range(gn):
                        b0 = 128 * c
                        nc.tensor.matmul(
                            g_ps[:], pT[:, b0:b0 + IC],
                            pT[:, b0 + IC:b0 + 2 * IC],
                            start=(c == 0), stop=False,
                            skip_group_check=True,
                        )
                        nc.tensor.matmul(
                            g_ps[:], pT[:, b0 + 2 * IC:b0 + 3 * IC],
                            pT[:, b0 + 3 * IC:b0 + 4 * IC],
                            start=False, stop=(c == gn - 1),
                            skip_group_check=True,
                        )
                    gt_sb = sbS.tile([IC, IC], FP32, tag="Gt")
                    nc.scalar.activation(
                        gt_sb[:], g_ps[:],
                        mybir.ActivationFunctionType.Copy,
                        scale=1.0 / (N * SCALE * SCALE),
                    )
                    nc.tensor.matmul(v_ps, gt_sb[:], thb_sb,
                                     start=(gi == 0), stop=(gi == NG - 1),
                                     skip_group_check=True)
                    m2_ps = psG.tile([IC, C], FP32, tag="G")
                    nc.tensor.matmul(m2_ps[:], gt_sb[:], thw_sb,
                                     start=True, stop=True,
                                     skip_group_check=True)
                    m2_sb = sbS.tile([IC, C], FP32, tag="m2sb")
                    nc.scalar.activation(
                        m2_sb[:], m2_ps[:], mybir.ActivationFunctionType.Copy
                    )
                    for cpos in (0, 64):
                        nc.tensor.matmul(
                            w4_ps[cpos:cpos + C, :], m2_sb[:], wwT_sb,
                            start=(gi == 0), stop=(gi == NG - 1 and cpos == 64),
                            tile_position=(0, cpos),
                            skip_group_check=True,
                        )

                # 512-col slice -> (refs tile, col offset) map
                smap = []
                for k, w in enumerate(RSPLIT):
                    for off in range(0, w, 512):
                        smap.append((k, off))

                # process 2048 refs cols (2 pooled blocks) per PSUM tile:
                # one double-width DVE reduce per 4 conv matmuls
                for u in range(8):
                    cp = psA.tile([128, 1024], FP32, tag="conv")
                    for s in range(4):
                        k, off = smap[4 * u + s]
                        rt = refs_t[k]
                        # block-diag pgw gives (phi; g) rows from
                        # (ref; ref_align) partitions in one matmul
                        nc.tensor.matmul(
                            cp[64 * (s % 2):64 * (s % 2) + 64,
                               512 * (s // 2):512 * (s // 2) + 512],
                            pgw_sb[:, 0:C], rt[:, off:off + 512],
                            start=True, stop=True,
                            tile_position=(0, 64 * (s % 2)),
                        )
                    # fused 2x2 maxpool over both blocks: PSUM f32 -> bf16
                    po = pooled[:, u * 256:(u + 1) * 256]
                    nc.vector.reduce_max(
                        po.rearrange("p (q w) -> p q w", q=4),
                        cp[:].rearrange("p (q h0 w w0) -> p q w h0 w0",
                                        q=4, h0=2, w=W // 2, w0=2),
                        axis=mybir.AxisListType.XY,
                    )
                    # transpose pipeline runs one unit (2 blocks) behind
                    if u >= 1:
                        for prev in (2 * u - 2, 2 * u - 1):
                            emit_transpose(prev)
                            gi = grp_of[prev]
                            if prev == GROUPS[gi][0] + GROUPS[gi][1] - 1:
                                emit_group_tail(gi)
                    if u == 2:
                        # refs stream is ~half issued; prefetch target now
                        for tq in range(2):
                            nc.scalar.dma_start(
                                out=tgt_tiles[tq][:],
                                in_=tgt[tq * 128:(tq + 1) * 128, :],
                            )

                for prev in (14, 15):
                    emit_transpose(prev)
                emit_group_tail(NG - 1)

                # block-diag (I + correction)^T on DVE (idle at drain)
                nc.vector.scalar_tensor_tensor(
                    out=w4bd[0:C, 0:C], in0=w4_ps[0:C, :], scalar=1.0,
                    in1=idbf_sb[0:C, 0:C],
                    op0=mybir.AluOpType.mult, op1=mybir.AluOpType.add,
                )
                nc.vector.scalar_tensor_tensor(
                    out=w4bd[C:128, C:128], in0=w4_ps[C:128, :], scalar=1.0,
                    in1=idbf_sb[0:C, 0:C],
                    op0=mybir.AluOpType.mult, op1=mybir.AluOpType.add,
                )
                v_sb = pers.tile([IC, 1], FP32, tag="vsb")
                nc.scalar.activation(
                    v_sb[:], v_ps, mybir.ActivationFunctionType.Copy
                )
                # b2 as a per-partition column, duplicated on partitions 64:128
                b2c_ps = psG.tile([128, 1], FP32, tag="G")
                for cpos in (0, 64):
                    nc.tensor.matmul(
                        b2c_ps[cpos:cpos + C, :], wwT_sb, v_sb[:],
                        start=True, stop=False, tile_position=(0, cpos),
                        skip_group_check=True,
                    )
                    nc.tensor.matmul(
                        b2c_ps[cpos:cpos + C, :], wbr_sb[:], one_sb[:, :],
                        start=False, stop=True, tile_position=(0, cpos),
                        skip_group_check=True,
                    )
                b2c_sb = pers.tile([128, 1], FP32, tag="b2csb")
                nc.scalar.activation(
                    b2c_sb[:], b2c_ps[:], mybir.ActivationFunctionType.Copy
                )

            # ---------- Phase D: fused final conv over target (bf16) ----------
            with tc.tile_pool(name="psD", bufs=4, space="PSUM") as psD, \
                 tc.tile_pool(name="outp", bufs=2) as sbO:
                for w in range(2):
                    tt = tgt_tiles[w]
                    ot = sbO.tile([128, OCHUNK], BF16, tag="out")
                    for i in range(OCHUNK // 512):
                        op = psD.tile([128, 512], FP32, tag="od")
                        tsl = slice(i * 512, (i + 1) * 512)
                        nc.tensor.matmul(
                            op[:], w4bd[:], tt[:, tsl],
                            start=True, stop=True,
                        )
                        # bias-add + bf16 cast, alternating DVE / ACT
                        isl = slice(i * 512, (i + 1) * 512)
                        if i % 2 == 0:
                            nc.vector.tensor_scalar_add(
                                ot[:, isl], op[:], b2c_sb[:]
                            )
                        else:
                            nc.scalar.activation(
                                ot[:, isl], op[:],
                                mybir.ActivationFunctionType.Identity,
                                bias=b2c_sb[:],
                            )
                    eng = nc.sync if w % 2 == 0 else nc.scalar
                    eng.dma_start(
                        out=out[w * 128:(w + 1) * 128, :], in_=ot[:]
                    )

    nc.compile()
    return nc


def _in_maps(target, ref, ref_align, theta_w, theta_b, phi_w, phi_b,
             g_w, g_b, W_w, W_b):
    f32, bf16 = np.float32, ml_dtypes.bfloat16
    fp8 = ml_dtypes.float8_e4m3
    wBv = np.zeros((IC, 2 * C + 1), dtype=f32)
    wBv[:, 0:C] = theta_w
    wBv[:, C:2 * C] = W_w.T
    wBv[:, 2 * C] = theta_b
    # block-diag stationary: rows 0:64 -> phi_w.T in cols 0:32,
    # rows 64:128 -> g_w.T in cols 32:64
    pgwbd = np.zeros((128, C), dtype=f32)
    pgwbd[0:C, 0:IC] = phi_w.T * SCALE
    pgwbd[C:128, IC:C] = g_w.T * SCALE
    bias512 = np.tile(np.concatenate([phi_b, g_b]) * SCALE, 8).reshape(1, 512)
    common = {
        "pgw": pgwbd.astype(fp8),
        "wB": wBv,
        "wbr": W_b.reshape(1, C).astype(f32),
        "biasT": np.broadcast_to(bias512, (128, 512)).astype(bf16),
    }
    maps = []
    for core in range(8):
        b, u = core // 2, core % 2
        refs = np.concatenate(
            [ref[b].reshape(C, N), ref_align[b].reshape(C, N)], axis=0
        ).astype(fp8)
        # chunk-major repack, padded to 2048 cols per chunk row-block
        refsc = np.zeros((len(RSPLIT) * 128, 2048), dtype=fp8)
        col = 0
        for k, wd in enumerate(RSPLIT):
            refsc[k * 128:(k + 1) * 128, 0:wd] = refs[:, col:col + wd]
            col += wd
        th = target[b, :, u * (H // 2):(u + 1) * (H // 2), :].reshape(C, NH)
        tgtv = np.concatenate([th[:, :NH // 2], th[:, NH // 2:]], axis=0)
        tgtc = np.concatenate(
            [tgtv[:, q * TCHUNK:(q + 1) * TCHUNK] for q in range(2)], axis=0
        ).astype(bf16)
        maps.append({"refs": refsc,
                     "tgt": np.ascontiguousarray(tgtc), **common})
    return maps


def _gather(res) -> np.ndarray:
    out = np.empty((B, C, H, W), dtype=np.float32)
    for core in range(8):
        o = np.asarray(res.results[core]["o"]).astype(np.float32)
        # undo chunk-major: [2*128, 2048] -> [128, 4096] -> [64, 8192]
        o = np.concatenate([o[w * 128:(w + 1) * 128, :] for w in range(2)],
                           axis=1)
        half = np.concatenate([o[:C, :], o[C:, :]], axis=1)  # [64, 8192]
        b, u = core // 2, core % 2
        out[b, :, u * (H // 2):(u + 1) * (H // 2), :] = half.reshape(C, H // 2, W)
    return out


def kernel(**inputs) -> np.ndarray:
    if "nc" not in _CACHED:
        _CACHED["nc"] = _build_program()
    nc = _CACHED["nc"]
    maps = _in_maps(**inputs)
    res = run_bass_kernel_spmd(nc, maps, list(range(8)))
    return _gather(res)


# revision 25
# speedup vs baseline: 1.1422x; 1.0690x over previous
"""NonLocal block kernel for 8 Trainium2 NeuronCores.

Algebraic restructuring: the softmax-free attention

    s = theta^T phi / N ;  y = s . g^T   (per batch)

is reassociated as y = (G/N) @ theta with G[i,j] = sum_m g[i,m] phi[j,m]
(a [32,32] matrix per batch).  Folding the surrounding 1x1 convs:

    out = (I + W_w (G/N) theta_w) @ target + (W_w (G/N) theta_b + W_b)

so after G is known the whole module is one 64x64 1x1-conv over target.

Sharding: batch b -> core pair (2b, 2b+1); each core of the pair computes
G for its batch redundantly (reads full ref/ref_align for the batch) and
produces half of the spatial output (no cross-core communication).

Precision: tolerance is 2e-2 rel; the G path only perturbs the output at
the ~1e-3 level, so refs and the phi/g conv run in fp8e4 (weights scaled
x16 to dodge denormals; the x256 on G is divided out once).  target, the
final conv, and the output are bf16 (~0.4% rel), PSUM accumulation f32.
W4 is accumulated as the pure correction (W4 - I)^T; the identity is
re-added during the bf16 block-diag build (diag correction ~1e-4 is below
bf16 ulp either way).

Engine split: PE does convs/transposes/G (block-diagonal stationaries
fuse phi+g and both u-halves into single full-depth matmuls), DVE does
only the 2x2 maxpool reduces plus half the output bias-adds, ACT adds
the phi/g biases onto pooled (per-partition bias), evacuates transposed
groups from PSUM, and does the other half of output bias-adds.

All DRAM tensors are chunk-major (host repacked) so every DMA is one
fully contiguous block; all input DMAs are issued up-front to
persistent tiles across the SP/ACT/Pool queues.
"""

import sys

for _p in ("/opt/trn_rl_repo",):
    if _p not in sys.path:
        sys.path.insert(0, _p)

import ml_dtypes
import numpy as np

import concourse.bass as bass
import concourse.mybir as mybir
from concourse import bacc
import concourse.tile as tile
from concourse.masks import make_identity
from concourse.bass_utils import run_bass_kernel_spmd

B, C, IC, H, W = 4, 64, 32, 128, 128
N = H * W            # 16384
NH = N // 2          # spatial positions per core (half batch)
M = N // 4           # 4096 pooled positions per batch
FP32 = mybir.dt.float32
BF16 = mybir.dt.bfloat16
FP8 = mybir.dt.float8e4
SCALE = 16.0         # host scales pgw and pgb by this; /SCALE^2 at gt copy

# refs chunk column sizes (chunk-major in DRAM, one contiguous DMA each);
# front chunks are small so the first conv can start early
RSPLIT = (1024, 1024, 2048, 2048, 2048, 2048, 2048, 2048, 2048)
assert sum(RSPLIT) == N
TCHUNK = 2048        # tgt cols per chunk (bf16 -> 512 KiB contiguous)
OCHUNK = 2048        # out cols per write (bf16 -> 512 KiB contiguous)
# pooled-block grouping for transpose/G/W4 streaming: small tail groups
# shorten the post-stream drain
GROUPS = ((0, 4), (4, 4), (8, 4), (12, 2), (14, 2))

_CACHED = {}


def _build_program() -> bass.Bass:
    nc = bacc.Bacc("TRN2", target_bir_lowering=False, debug=False)

    refs = nc.dram_tensor("refs", [len(RSPLIT) * 128, 2048], FP8,
                          kind="ExternalInput")
    tgt = nc.dram_tensor("tgt", [2 * 128, TCHUNK], BF16, kind="ExternalInput")
    pgw = nc.dram_tensor("pgw", [128, C], FP8, kind="ExternalInput")
    wB = nc.dram_tensor("wB", [IC, 2 * C + 1], FP32, kind="ExternalInput")
    wbr = nc.dram_tensor("wbr", [1, C], FP32, kind="ExternalInput")
    biasT = nc.dram_tensor("biasT", [128, 512], BF16, kind="ExternalInput")
    out = nc.dram_tensor("o", [2 * 128, OCHUNK], BF16, kind="ExternalOutput")

    with tile.TileContext(nc) as tc, \
         nc.allow_low_precision("bf16/fp8 path well within 2e-2 tolerance"):
        with (
            tc.tile_pool(name="const", bufs=1) as cpool,
            tc.tile_pool(name="small", bufs=2) as sbS,
            tc.tile_pool(name="persist", bufs=1) as pers,
        ):
            # ---- all input DMAs issued up-front; every destination is
            # persistent so no trigger ever waits on a buffer recycle ----
            pgw_sb = cpool.tile([128, C], FP8, tag="pgw")
            refs_t = []
            for k, w in enumerate(RSPLIT):
                t = pers.tile([128, w], FP8, tag=f"refs{k}")
                refs_t.append(t)
            # all refs strictly FIFO on the sync queue: a single queue still
            # saturates the fabric, and FIFO order means chunk k completes
            # before k+1 starts, so the conv is never starved by later bulk
            nc.sync.dma_start(out=pgw_sb[:], in_=pgw[:])
            for k in range(len(RSPLIT)):
                nc.sync.dma_start(
                    out=refs_t[k][:],
                    in_=refs[k * 128:(k + 1) * 128, 0:RSPLIT[k]],
                )
            wB_sb = cpool.tile([IC, 2 * C + 1], FP32, tag="wB")
            nc.scalar.dma_start(out=wB_sb[:], in_=wB[:])
            wbr_sb = cpool.tile([1, C], FP32, tag="wbr")
            nc.scalar.dma_start(out=wbr_sb[:], in_=wbr[:])
            biasT_sb = cpool.tile([128, 512], BF16, tag="biasT")
            nc.scalar.dma_start(out=biasT_sb[:], in_=biasT[:])
            # gpsimd: identity build first (transposes need it early),
            # then the target prefetch
            idbf_sb = cpool.tile([128, 128], BF16, tag="identb")
            make_identity(nc, idbf_sb[:])
            one_sb = cpool.tile([1, 1], FP32, tag="one")
            nc.gpsimd.memset(one_sb[:], 1.0)
            w4bd = pers.tile([128, 128], BF16, tag="w4bd")
            nc.gpsimd.memset(w4bd[:], 0.0)
            # tgt tiles allocated here; their DMAs are issued mid-stream
            # (on the scalar queue) so they don't steal refs bandwidth
            tgt_tiles = []
            for tq in range(2):
                tt = pers.tile([128, TCHUNK], BF16, tag=f"tgt{tq}")
                tgt_tiles.append(tt)

            thw_sb = wB_sb[:, 0:C]
            wwT_sb = wB_sb[:, C:2 * C]
            thb_sb = wB_sb[:, 2 * C:2 * C + 1]

            # pooled conv outputs (bf16, x16 scaled, +bias), 4-way stacked
            pooled = pers.tile([128, 16 * 128], BF16, tag="pooled")
            # transposed pooled, bf16
            phigT = pers.tile([128, 16 * 128], BF16, tag="phigT")

            # map block index -> group index
            grp_of = {}
            for gi, (g0, gn) in enumerate(GROUPS):
                for b_ in range(g0, g0 + gn):
                    grp_of[b_] = gi
            NG = len(GROUPS)

            # ---- Phase A: fused fp8 convs + DVE 2x2 maxpool + ACT bias,
            # with transpose/G/W4 accumulation streamed per group ----
            with tc.tile_pool(name="psA", bufs=2, space="PSUM") as psA, \
                 tc.tile_pool(name="psB", bufs=2, space="PSUM") as psB, \
                 tc.tile_pool(name="psG", bufs=1, space="PSUM") as psG, \
                 tc.tile_pool(name="psW", bufs=1, space="PSUM") as psW:
                # W4 correction [cols 0:64] and v=G.theta_b [col 64] share
                # one PSUM bank (disjoint regions, independent accumulation)
                wv_ps = psW.tile([128, C + 1], FP32, tag="wv")
                w4_ps = wv_ps[:, 0:C]
                v_ps = wv_ps[0:IC, C:C + 1]
                tpp_box = [None]

                def emit_transpose(blk):
                    gi = grp_of[blk]
                    g0, gn = GROUPS[gi]
                    if blk == g0:
                        tpp_new = psB.tile([128, 128 * gn], BF16, tag="tp")
                        tpp_box[0] = tpp_new
                    bi = blk - g0
                    nc.tensor.matmul(
                        tpp_box[0][:, 128 * bi:128 * (bi + 1)],
                        pooled[:, 128 * blk:128 * (blk + 1)],
                        idbf_sb[:],
                        is_transpose=True, start=True, stop=True,
                        skip_group_check=True,
                    )

                def emit_group_tail(gi):
                    g0, gn = GROUPS[gi]
                    pT = phigT[:, 128 * g0:128 * (g0 + gn)]
                    # transposed group -> bf16 SBUF, phi/g bias fused into
                    # the copy (bias commutes with maxpool and transpose)
                    nc.vector.scalar_tensor_tensor(
                        out=pT, in0=tpp_box[0][:], scalar=1.0,
                        in1=biasT_sb[:, 0:128 * gn],
                        op0=mybir.AluOpType.mult, op1=mybir.AluOpType.add,
                    )
                    g_ps = psG.tile([IC, IC], FP32, tag="G")
                    for c in range(gn):
                        b0 = 128 * c
                        nc.tensor.matmul(
                            g_ps[:], pT[:, b0:b0 + IC],
                            pT[:, b0 + IC:b0 + 2 * IC],
                            start=(c == 0), stop=False,
                            skip_group_check=True,
                        )
                        nc.tensor.matmul(
                            g_ps[:], pT[:, b0 + 2 * IC:b0 + 3 * IC],
                            pT[:, b0 + 3 * IC:b0 + 4 * IC],
                            start=False, stop=(c == gn - 1),
                            skip_group_check=True,
                        )
                    gt_sb = sbS.tile([IC, IC], FP32, tag="Gt")
                    nc.scalar.activation(
                        gt_sb[:], g_ps[:],
                        mybir.ActivationFunctionType.Copy,
                        scale=1.0 / (N * SCALE * SCALE),
                    )
                    nc.tensor.matmul(v_ps, gt_sb[:], thb_sb,
                                     start=(gi == 0), stop=(gi == NG - 1),
                                     skip_group_check=True)
                    m2_ps = psG.tile([IC, C], FP32, tag="G")
                    nc.tensor.matmul(m2_ps[:], gt_sb[:], thw_sb,
                                     start=True, stop=True,
                                     skip_group_check=True)
                    m2_sb = sbS.tile([IC, C], FP32, tag="m2sb")
                    nc.scalar.activation(
                        m2_sb[:], m2_ps[:], mybir.ActivationFunctionType.Copy
                    )
                    for cpos in (0, 64):
                        nc.tensor.matmul(
                            w4_ps[cpos:cpos + C, :], m2_sb[:], wwT_sb,
                            start=(gi == 0), stop=(gi == NG - 1 and cpos == 64),
                            tile_position=(0, cpos),
                            skip_group_check=True,
                        )

                # 512-col slice -> (refs tile, col offset) map
                smap = []
                for k, w in enumerate(RSPLIT):
                    for off in range(0, w, 512):
                        smap.append((k, off))

                # process 2048 refs cols (2 pooled blocks) per PSUM tile:
                # one double-width DVE reduce per 4 conv matmuls
                for u in range(8):
                    cp = psA.tile([128, 1024], FP32, tag="conv")
                    for s in range(4):
                        k, off = smap[4 * u + s]
                        rt = refs_t[k]
                        # block-diag pgw gives (phi; g) rows from
                        # (ref; ref_align) partitions in one matmul
                        nc.tensor.matmul(
                            cp[64 * (s % 2):64 * (s % 2) + 64,
                               512 * (s // 2):512 * (s // 2) + 512],
                            pgw_sb[:, 0:C], rt[:, off:off + 512],
                            start=True, stop=True,
                            tile_position=(0, 64 * (s % 2)),
                        )
                    # fused 2x2 maxpool over both blocks: PSUM f32 -> bf16
                    po = pooled[:, u * 256:(u + 1) * 256]
                    nc.vector.reduce_max(
                        po.rearrange("p (q w) -> p q w", q=4),
                        cp[:].rearrange("p (q h0 w w0) -> p q w h0 w0",
                                        q=4, h0=2, w=W // 2, w0=2),
                        axis=mybir.AxisListType.XY,
                    )
                    # transpose pipeline runs one unit (2 blocks) behind
                    if u >= 1:
                        for prev in (2 * u - 2, 2 * u - 1):
                            emit_transpose(prev)
                            gi = grp_of[prev]
                            if prev == GROUPS[gi][0] + GROUPS[gi][1] - 1:
                                emit_group_tail(gi)
                    if u == 2:
                        # refs stream is ~half issued; prefetch target now
                        for tq in range(2):
                            nc.scalar.dma_start(
                                out=tgt_tiles[tq][:],
                                in_=tgt[tq * 128:(tq + 1) * 128, :],
                            )

                for prev in (14, 15):
                    emit_transpose(prev)
                emit_group_tail(NG - 1)

                # block-diag (I + correction)^T on DVE (idle at drain)
                nc.vector.scalar_tensor_tensor(
                    out=w4bd[0:C, 0:C], in0=w4_ps[0:C, :], scalar=1.0,
                    in1=idbf_sb[0:C, 0:C],
                    op0=mybir.AluOpType.mult, op1=mybir.AluOpType.add,
                )
                nc.vector.scalar_tensor_tensor(
                    out=w4bd[C:128, C:128], in0=w4_ps[C:128, :], scalar=1.0,
                    in1=idbf_sb[0:C, 0:C],
                    op0=mybir.AluOpType.mult, op1=mybir.AluOpType.add,
                )
                v_sb = pers.tile([IC, 1], FP32, tag="vsb")
                nc.scalar.activation(
                    v_sb[:], v_ps, mybir.ActivationFunctionType.Copy
                )

            # ---------- Phase D: fused final conv over target (bf16) ----------
            with tc.tile_pool(name="psD", bufs=4, space="PSUM") as psD, \
                 tc.tile_pool(name="psE", bufs=1, space="PSUM") as psE, \
                 tc.tile_pool(name="outp", bufs=2) as sbO:
                b2c_box = [None]
                b2c_sb = pers.tile([128, 1], FP32, tag="b2csb")

                def emit_b2c():
                    # b2 column (dup'd on partitions 64:128); emitted after
                    # the first final-conv matmul so it is off the W4->conv
                    # critical path
                    b2c_ps = psE.tile([128, 1], FP32, tag="b2")
                    for cpos in (0, 64):
                        nc.tensor.matmul(
                            b2c_ps[cpos:cpos + C, :], wwT_sb, v_sb[:],
                            start=True, stop=False, tile_position=(0, cpos),
                            skip_group_check=True,
                        )
                        nc.tensor.matmul(
                            b2c_ps[cpos:cpos + C, :], wbr_sb[:], one_sb[:, :],
                            start=False, stop=True, tile_position=(0, cpos),
                            skip_group_check=True,
                        )
                    nc.scalar.activation(
                        b2c_sb[:], b2c_ps[:], mybir.ActivationFunctionType.Copy
                    )
                    b2c_box[0] = True

                for w in range(2):
                    tt = tgt_tiles[w]
                    ot = sbO.tile([128, OCHUNK], BF16, tag="out")
                    for i in range(OCHUNK // 512):
                        op = psD.tile([128, 512], FP32, tag="od")
                        tsl = slice(i * 512, (i + 1) * 512)
                        nc.tensor.matmul(
                            op[:], w4bd[:], tt[:, tsl],
                            start=True, stop=True,
                        )
                        if b2c_box[0] is None:
                            emit_b2c()
                        # bias-add + bf16 cast, alternating DVE / ACT
                        isl = slice(i * 512, (i + 1) * 512)
                        if i % 2 == 0:
                            nc.vector.tensor_scalar_add(
                                ot[:, isl], op[:], b2c_sb[:]
                            )
                        else:
                            nc.scalar.activation(
                                ot[:, isl], op[:],
                                mybir.ActivationFunctionType.Identity,
                                bias=b2c_sb[:],
                            )
                    eng = nc.sync if w % 2 == 0 else nc.scalar
                    eng.dma_start(
                        out=out[w * 128:(w + 1) * 128, :], in_=ot[:]
                    )

    nc.compile()
    return nc


def _in_maps(target, ref, ref_align, theta_w, theta_b, phi_w, phi_b,
             g_w, g_b, W_w, W_b):
    f32, bf16 = np.float32, ml_dtypes.bfloat16
    fp8 = ml_dtypes.float8_e4m3
    wBv = np.zeros((IC, 2 * C + 1), dtype=f32)
    wBv[:, 0:C] = theta_w
    wBv[:, C:2 * C] = W_w.T
    wBv[:, 2 * C] = theta_b
    # block-diag stationary: rows 0:64 -> phi_w.T in cols 0:32,
    # rows 64:128 -> g_w.T in cols 32:64
    pgwbd = np.zeros((128, C), dtype=f32)
    pgwbd[0:C, 0:IC] = phi_w.T * SCALE
    pgwbd[C:128, IC:C] = g_w.T * SCALE
    bias512 = np.tile(np.concatenate([phi_b, g_b]) * SCALE, 8).reshape(1, 512)
    common = {
        "pgw": pgwbd.astype(fp8),
        "wB": wBv,
        "wbr": W_b.reshape(1, C).astype(f32),
        "biasT": np.broadcast_to(bias512, (128, 512)).astype(bf16),
    }
    maps = []
    for core in range(8):
        b, u = core // 2, core % 2
        refs = np.concatenate(
            [ref[b].reshape(C, N), ref_align[b].reshape(C, N)], axis=0
        ).astype(fp8)
        # chunk-major repack, padded to 2048 cols per chunk row-block
        refsc = np.zeros((len(RSPLIT) * 128, 2048), dtype=fp8)
        col = 0
        for k, wd in enumerate(RSPLIT):
            refsc[k * 128:(k + 1) * 128, 0:wd] = refs[:, col:col + wd]
            col += wd
        th = target[b, :, u * (H // 2):(u + 1) * (H // 2), :].reshape(C, NH)
        tgtv = np.concatenate([th[:, :NH // 2], th[:, NH // 2:]], axis=0)
        tgtc = np.concatenate(
            [tgtv[:, q * TCHUNK:(q + 1) * TCHUNK] for q in range(2)], axis=0
        ).astype(bf16)
        maps.append({"refs": refsc,
                     "tgt": np.ascontiguousarray(tgtc), **common})
    return maps


def _gather(res) -> np.ndarray:
    out = np.empty((B, C, H, W), dtype=np.float32)
    for core in range(8):
        o = np.asarray(res.results[core]["o"]).astype(np.float32)
        # undo chunk-major: [2*128, 2048] -> [128, 4096] -> [64, 8192]
        o = np.concatenate([o[w * 128:(w + 1) * 128, :] for w in range(2)],
                           axis=1)
        half = np.concatenate([o[:C, :], o[C:, :]], axis=1)  # [64, 8192]
        b, u = core // 2, core % 2
        out[b, :, u * (H // 2):(u + 1) * (H // 2), :] = half.reshape(C, H // 2, W)
    return out


def kernel(**inputs) -> np.ndarray:
    if "nc" not in _CACHED:
        _CACHED["nc"] = _build_program()
    nc = _CACHED["nc"]
    maps = _in_maps(**inputs)
    res = run_bass_kernel_spmd(nc, maps, list(range(8)))
    return _gather(res)


# revision 26
# speedup vs baseline: 1.1731x; 1.0270x over previous
"""NonLocal block kernel for 8 Trainium2 NeuronCores.

Algebraic restructuring: the softmax-free attention

    s = theta^T phi / N ;  y = s . g^T   (per batch)

is reassociated as y = (G/N) @ theta with G[i,j] = sum_m g[i,m] phi[j,m]
(a [32,32] matrix per batch).  Folding the surrounding 1x1 convs:

    out = (I + W_w (G/N) theta_w) @ target + (W_w (G/N) theta_b + W_b)

so after G is known the whole module is one 64x64 1x1-conv over target.

Sharding: batch b -> core pair (2b, 2b+1); each core of the pair computes
G for its batch redundantly (reads full ref/ref_align for the batch) and
produces half of the spatial output (no cross-core communication).

Precision: tolerance is 2e-2 rel; the G path only perturbs the output at
the ~1e-3 level, so refs and the phi/g conv run in fp8e4 (weights scaled
x16 to dodge denormals; the x256 on G is divided out once).  target, the
final conv, and the output are bf16 (~0.4% rel), PSUM accumulation f32.
W4 is accumulated as the pure correction (W4 - I)^T; the identity is
re-added during the bf16 block-diag build (diag correction ~1e-4 is below
bf16 ulp either way).

Engine split: PE does convs/transposes/G (block-diagonal stationaries
fuse phi+g and both u-halves into single full-depth matmuls), DVE does
only the 2x2 maxpool reduces plus half the output bias-adds, ACT adds
the phi/g biases onto pooled (per-partition bias), evacuates transposed
groups from PSUM, and does the other half of output bias-adds.

All DRAM tensors are chunk-major (host repacked) so every DMA is one
fully contiguous block; all input DMAs are issued up-front to
persistent tiles across the SP/ACT/Pool queues.
"""

import sys

for _p in ("/opt/trn_rl_repo",):
    if _p not in sys.path:
        sys.path.insert(0, _p)

import ml_dtypes
import numpy as np

import concourse.bass as bass
import concourse.mybir as mybir
from concourse import bacc
import concourse.tile as tile
from concourse.masks import make_identity
from concourse.bass_utils import run_bass_kernel_spmd

B, C, IC, H, W = 4, 64, 32, 128, 128
N = H * W            # 16384
NH = N // 2          # spatial positions per core (half batch)
M = N // 4           # 4096 pooled positions per batch
FP32 = mybir.dt.float32
BF16 = mybir.dt.bfloat16
FP8 = mybir.dt.float8e4
SCALE = 16.0         # host scales pgw and pgb by this; /SCALE^2 at gt copy

# refs chunk column sizes (chunk-major in DRAM, one contiguous DMA each);
# front chunks are small so the first conv can start early
RSPLIT = (1024, 1024, 2048, 2048, 2048, 2048, 2048, 2048, 2048)
assert sum(RSPLIT) == N
TCHUNK = 2048        # tgt cols per chunk (bf16 -> 512 KiB contiguous)
OCHUNK = 2048        # out cols per write (bf16 -> 512 KiB contiguous)
# pooled-block grouping for transpose/G/W4 streaming: small tail groups
# shorten the post-stream drain
GROUPS = ((0, 4), (4, 4), (8, 4), (12, 2), (14, 2))

_CACHED = {}


def _build_program() -> bass.Bass:
    nc = bacc.Bacc("TRN2", target_bir_lowering=False, debug=False)

    refs = nc.dram_tensor("refs", [len(RSPLIT) * 128, 2048], FP8,
                          kind="ExternalInput")
    tgt = nc.dram_tensor("tgt", [2 * 128, TCHUNK], BF16, kind="ExternalInput")
    pgw = nc.dram_tensor("pgw", [128, C], FP8, kind="ExternalInput")
    wB = nc.dram_tensor("wB", [IC, 2 * C + 1], FP32, kind="ExternalInput")
    wbr = nc.dram_tensor("wbr", [1, C], FP32, kind="ExternalInput")
    biasT = nc.dram_tensor("biasT", [128, 512], BF16, kind="ExternalInput")
    out = nc.dram_tensor("o", [2 * 128, OCHUNK], BF16, kind="ExternalOutput")

    with tile.TileContext(nc) as tc, \
         nc.allow_low_precision("bf16/fp8 path well within 2e-2 tolerance"):
        with (
            tc.tile_pool(name="const", bufs=1) as cpool,
            tc.tile_pool(name="small", bufs=2) as sbS,
            tc.tile_pool(name="persist", bufs=1) as pers,
        ):
            # ---- all input DMAs issued up-front; every destination is
            # persistent so no trigger ever waits on a buffer recycle ----
            pgw_sb = cpool.tile([128, C], FP8, tag="pgw")
            refs_t = []
            for k, w in enumerate(RSPLIT):
                t = pers.tile([128, w], FP8, tag=f"refs{k}")
                refs_t.append(t)
            # all refs strictly FIFO on the sync queue: a single queue still
            # saturates the fabric, and FIFO order means chunk k completes
            # before k+1 starts, so the conv is never starved by later bulk
            nc.sync.dma_start(out=pgw_sb[:], in_=pgw[:])
            for k in range(len(RSPLIT)):
                nc.sync.dma_start(
                    out=refs_t[k][:],
                    in_=refs[k * 128:(k + 1) * 128, 0:RSPLIT[k]],
                )
            wB_sb = cpool.tile([IC, 2 * C + 1], FP32, tag="wB")
            nc.scalar.dma_start(out=wB_sb[:], in_=wB[:])
            wbr_sb = cpool.tile([1, C], FP32, tag="wbr")
            nc.scalar.dma_start(out=wbr_sb[:], in_=wbr[:])
            biasT_sb = cpool.tile([128, 512], BF16, tag="biasT")
            nc.scalar.dma_start(out=biasT_sb[:], in_=biasT[:])
            # gpsimd: identity build first (transposes need it early),
            # then the target prefetch
            idbf_sb = cpool.tile([128, 128], BF16, tag="identb")
            make_identity(nc, idbf_sb[:])
            one_sb = cpool.tile([1, 1], FP32, tag="one")
            nc.gpsimd.memset(one_sb[:], 1.0)
            w4bd = pers.tile([128, 128], BF16, tag="w4bd")
            nc.gpsimd.memset(w4bd[:], 0.0)
            # tgt tiles allocated here; their DMAs are issued mid-stream
            # (on the scalar queue) so they don't steal refs bandwidth
            tgt_tiles = []
            for tq in range(2):
                tt = pers.tile([128, TCHUNK], BF16, tag=f"tgt{tq}")
                tgt_tiles.append(tt)

            thw_sb = wB_sb[:, 0:C]
            wwT_sb = wB_sb[:, C:2 * C]
            thb_sb = wB_sb[:, 2 * C:2 * C + 1]

            # pooled conv outputs (bf16, x16 scaled, +bias), 4-way stacked
            pooled = pers.tile([128, 16 * 128], BF16, tag="pooled")
            # transposed pooled, bf16
            phigT = pers.tile([128, 16 * 128], BF16, tag="phigT")

            # map block index -> group index
            grp_of = {}
            for gi, (g0, gn) in enumerate(GROUPS):
                for b_ in range(g0, g0 + gn):
                    grp_of[b_] = gi
            NG = len(GROUPS)

            # ---- Phase A: fused fp8 convs + DVE 2x2 maxpool + ACT bias,
            # with transpose/G/W4 accumulation streamed per group ----
            with tc.tile_pool(name="psA", bufs=2, space="PSUM") as psA, \
                 tc.tile_pool(name="psB", bufs=1, space="PSUM") as psB, \
                 tc.tile_pool(name="psG", bufs=2, space="PSUM") as psG, \
                 tc.tile_pool(name="psW", bufs=1, space="PSUM") as psW:
                # W4 correction [cols 0:64] and v=G.theta_b [col 64] share
                # one PSUM bank (disjoint regions, independent accumulation)
                wv_ps = psW.tile([128, C + 1], FP32, tag="wv")
                w4_ps = wv_ps[:, 0:C]
                v_ps = wv_ps[0:IC, C:C + 1]
                tpp_box = [None]

                def emit_transpose(blk):
                    gi = grp_of[blk]
                    g0, gn = GROUPS[gi]
                    if blk == g0:
                        tpp_new = psB.tile([128, 128 * gn], BF16, tag="tp")
                        tpp_box[0] = tpp_new
                    bi = blk - g0
                    nc.tensor.matmul(
                        tpp_box[0][:, 128 * bi:128 * (bi + 1)],
                        pooled[:, 128 * blk:128 * (blk + 1)],
                        idbf_sb[:],
                        is_transpose=True, start=True, stop=True,
                        skip_group_check=True,
                    )

                def emit_group_tail(gi):
                    g0, gn = GROUPS[gi]
                    pT = phigT[:, 128 * g0:128 * (g0 + gn)]
                    # transposed group -> bf16 SBUF, phi/g bias fused into
                    # the copy (bias commutes with maxpool and transpose)
                    nc.vector.scalar_tensor_tensor(
                        out=pT, in0=tpp_box[0][:], scalar=1.0,
                        in1=biasT_sb[:, 0:128 * gn],
                        op0=mybir.AluOpType.mult, op1=mybir.AluOpType.add,
                    )
                    g_ps = psG.tile([IC, IC], FP32, tag="G")
                    for c in range(gn):
                        b0 = 128 * c
                        nc.tensor.matmul(
                            g_ps[:], pT[:, b0:b0 + IC],
                            pT[:, b0 + IC:b0 + 2 * IC],
                            start=(c == 0), stop=False,
                            skip_group_check=True,
                        )
                        nc.tensor.matmul(
                            g_ps[:], pT[:, b0 + 2 * IC:b0 + 3 * IC],
                            pT[:, b0 + 3 * IC:b0 + 4 * IC],
                            start=False, stop=(c == gn - 1),
                            skip_group_check=True,
                        )
                    gt_sb = sbS.tile([IC, IC], FP32, tag="Gt")
                    nc.scalar.activation(
                        gt_sb[:], g_ps[:],
                        mybir.ActivationFunctionType.Copy,
                        scale=1.0 / (N * SCALE * SCALE),
                    )
                    nc.tensor.matmul(v_ps, gt_sb[:], thb_sb,
                                     start=(gi == 0), stop=(gi == NG - 1),
                                     skip_group_check=True)
                    m2_ps = psG.tile([IC, C], FP32, tag="G")
                    nc.tensor.matmul(m2_ps[:], gt_sb[:], thw_sb,
                                     start=True, stop=True,
                                     skip_group_check=True)
                    m2_sb = sbS.tile([IC, C], FP32, tag="m2sb")
                    nc.scalar.activation(
                        m2_sb[:], m2_ps[:], mybir.ActivationFunctionType.Copy
                    )
                    for cpos in (0, 64):
                        nc.tensor.matmul(
                            w4_ps[cpos:cpos + C, :], m2_sb[:], wwT_sb,
                            start=(gi == 0), stop=(gi == NG - 1 and cpos == 64),
                            tile_position=(0, cpos),
                            skip_group_check=True,
                        )

                # 512-col slice -> (refs tile, col offset) map
                smap = []
                for k, w in enumerate(RSPLIT):
                    for off in range(0, w, 512):
                        smap.append((k, off))

                # process 2048 refs cols (2 pooled blocks) per PSUM tile:
                # one double-width DVE reduce per 4 conv matmuls
                for u in range(8):
                    cp = psA.tile([128, 1024], FP32, tag="conv")
                    for s in range(4):
                        k, off = smap[4 * u + s]
                        rt = refs_t[k]
                        # block-diag pgw gives (phi; g) rows from
                        # (ref; ref_align) partitions in one matmul
                        nc.tensor.matmul(
                            cp[64 * (s % 2):64 * (s % 2) + 64,
                               512 * (s // 2):512 * (s // 2) + 512],
                            pgw_sb[:, 0:C], rt[:, off:off + 512],
                            start=True, stop=True,
                            tile_position=(0, 64 * (s % 2)),
                        )
                    # fused 2x2 maxpool over both blocks: PSUM f32 -> bf16
                    po = pooled[:, u * 256:(u + 1) * 256]
                    nc.vector.reduce_max(
                        po.rearrange("p (q w) -> p q w", q=4),
                        cp[:].rearrange("p (q h0 w w0) -> p q w h0 w0",
                                        q=4, h0=2, w=W // 2, w0=2),
                        axis=mybir.AxisListType.XY,
                    )
                    # transpose pipeline runs one unit (2 blocks) behind
                    if u >= 1:
                        for prev in (2 * u - 2, 2 * u - 1):
                            emit_transpose(prev)
                            gi = grp_of[prev]
                            if prev == GROUPS[gi][0] + GROUPS[gi][1] - 1:
                                emit_group_tail(gi)
                    if u == 4:
                        # refs stream fully landed; prefetch target now
                        for tq in range(2):
                            nc.scalar.dma_start(
                                out=tgt_tiles[tq][:],
                                in_=tgt[tq * 128:(tq + 1) * 128, :],
                            )

                for prev in (14, 15):
                    emit_transpose(prev)
                emit_group_tail(NG - 1)

                # block-diag (I + correction)^T on DVE (idle at drain)
                nc.vector.scalar_tensor_tensor(
                    out=w4bd[0:C, 0:C], in0=w4_ps[0:C, :], scalar=1.0,
                    in1=idbf_sb[0:C, 0:C],
                    op0=mybir.AluOpType.mult, op1=mybir.AluOpType.add,
                )
                nc.vector.scalar_tensor_tensor(
                    out=w4bd[C:128, C:128], in0=w4_ps[C:128, :], scalar=1.0,
                    in1=idbf_sb[0:C, 0:C],
                    op0=mybir.AluOpType.mult, op1=mybir.AluOpType.add,
                )
                v_sb = pers.tile([IC, 1], FP32, tag="vsb")
                nc.scalar.activation(
                    v_sb[:], v_ps, mybir.ActivationFunctionType.Copy
                )

            # ---------- Phase D: fused final conv over target (bf16) ----------
            with tc.tile_pool(name="psD", bufs=4, space="PSUM") as psD, \
                 tc.tile_pool(name="psE", bufs=1, space="PSUM") as psE, \
                 tc.tile_pool(name="outp", bufs=2) as sbO:
                b2c_box = [None]
                b2c_sb = pers.tile([128, 1], FP32, tag="b2csb")

                def emit_b2c():
                    # b2 column (dup'd on partitions 64:128); emitted after
                    # the first final-conv matmul so it is off the W4->conv
                    # critical path
                    b2c_ps = psE.tile([128, 1], FP32, tag="b2")
                    for cpos in (0, 64):
                        nc.tensor.matmul(
                            b2c_ps[cpos:cpos + C, :], wwT_sb, v_sb[:],
                            start=True, stop=False, tile_position=(0, cpos),
                            skip_group_check=True,
                        )
                        nc.tensor.matmul(
                            b2c_ps[cpos:cpos + C, :], wbr_sb[:], one_sb[:, :],
                            start=False, stop=True, tile_position=(0, cpos),
                            skip_group_check=True,
                        )
                    nc.scalar.activation(
                        b2c_sb[:], b2c_ps[:], mybir.ActivationFunctionType.Copy
                    )
                    b2c_box[0] = True

                for w in range(2):
                    tt = tgt_tiles[w]
                    ot = sbO.tile([128, OCHUNK], BF16, tag="out")
                    for i in range(OCHUNK // 512):
                        op = psD.tile([128, 512], FP32, tag="od")
                        tsl = slice(i * 512, (i + 1) * 512)
                        nc.tensor.matmul(
                            op[:], w4bd[:], tt[:, tsl],
                            start=True, stop=True,
                        )
                        if b2c_box[0] is None:
                            emit_b2c()
                        # bias-add + bf16 cast, alternating DVE / ACT
                        isl = slice(i * 512, (i + 1) * 512)
                        if i % 2 == 0:
                            nc.vector.tensor_scalar_add(
                                ot[:, isl], op[:], b2c_sb[:]
                            )
                        else:
                            nc.scalar.activation(
                                ot[:, isl], op[:],
                                mybir.ActivationFunctionType.Identity,
                                bias=b2c_sb[:],
                            )
                    eng = nc.sync if w % 2 == 0 else nc.scalar
                    eng.dma_start(
                        out=out[w * 128:(w + 1) * 128, :], in_=ot[:]
                    )

    nc.compile()
    return nc


def _in_maps(target, ref, ref_align, theta_w, theta_b, phi_w, phi_b,
             g_w, g_b, W_w, W_b):
    f32, bf16 = np.float32, ml_dtypes.bfloat16
    fp8 = ml_dtypes.float8_e4m3
    wBv = np.zeros((IC, 2 * C + 1), dtype=f32)
    wBv[:, 0:C] = theta_w
    wBv[:, C:2 * C] = W_w.T
    wBv[:, 2 * C] = theta_b
    # block-diag stationary: rows 0:64 -> phi_w.T in cols 0:32,
    # rows 64:128 -> g_w.T in cols 32:64
    pgwbd = np.zeros((128, C), dtype=f32)
    pgwbd[0:C, 0:IC] = phi_w.T * SCALE
    pgwbd[C:128, IC:C] = g_w.T * SCALE
    bias512 = np.tile(np.concatenate([phi_b, g_b]) * SCALE, 8).reshape(1, 512)
    common = {
        "pgw": pgwbd.astype(fp8),
        "wB": wBv,
        "wbr": W_b.reshape(1, C).astype(f32),
        "biasT": np.broadcast_to(bias512, (128, 512)).astype(bf16),
    }
    maps = []
    for core in range(8):
        b, u = core // 2, core % 2
        refs = np.concatenate(
            [ref[b].reshape(C, N), ref_align[b].reshape(C, N)], axis=0
        ).astype(fp8)
        # chunk-major repack, padded to 2048 cols per chunk row-block
        refsc = np.zeros((len(RSPLIT) * 128, 2048), dtype=fp8)
        col = 0
        for k, wd in enumerate(RSPLIT):
            refsc[k * 128:(k + 1) * 128, 0:wd] = refs[:, col:col + wd]
            col += wd
        th = target[b, :, u * (H // 2):(u + 1) * (H // 2), :].reshape(C, NH)
        tgtv = np.concatenate([th[:, :NH // 2], th[:, NH // 2:]], axis=0)
        tgtc = np.concatenate(
            [tgtv[:, q * TCHUNK:(q + 1) * TCHUNK] for q in range(2)], axis=0
        ).astype(bf16)
        maps.append({"refs": refsc,
                     "tgt": np.ascontiguousarray(tgtc), **common})
    return maps


def _gather(res) -> np.ndarray:
    out = np.empty((B, C, H, W), dtype=np.float32)
    for core in range(8):
        o = np.asarray(res.results[core]["o"]).astype(np.float32)
        # undo chunk-major: [2*128, 2048] -> [128, 4096] -> [64, 8192]
        o = np.concatenate([o[w * 128:(w + 1) * 128, :] for w in range(2)],
                           axis=1)
        half = np.concatenate([o[:C, :], o[C:, :]], axis=1)  # [64, 8192]
        b, u = core // 2, core % 2
        out[b, :, u * (H // 2):(u + 1) * (H // 2), :] = half.reshape(C, H // 2, W)
    return out


def kernel(**inputs) -> np.ndarray:
    if "nc" not in _CACHED:
        _CACHED["nc"] = _build_program()
    nc = _CACHED["nc"]
    maps = _in_maps(**inputs)
    res = run_bass_kernel_spmd(nc, maps, list(range(8)))
    return _gather(res)
